# revision 11
# baseline (speedup 1.0000x reference)
"""Adaptive Huber/MSE/L1 loss on 8 TRN2 NeuronCores (Bass/Tile), v4.

Reference math (per sample, N = 4,096,000 elements):
    e   = pred - true
    L2  = mean(e^2);  L1 = mean(|e|)
    huber = (S2 - SR) * 0.5 / N     (S2 = sum e^2, SR = sum relu(|e|-5)^2)
    use_l2 = (L2 <= 1) | (L2 < L1^2)
    loss = mean_over_batch(where(use_l2, L2, huber))

Sharding: data-parallel, sample i -> core i (32.8 MB of f32 in per core).
Each core emits a [1,5] row (S2, SR_act, SR_dve, S1_main, S1_tail); the
host finishes the branch math during unshard.

Measured facts driving the layout (HW traces, this session):
  - DMA: ~425-430 GB/s/core aggregate regardless of 8 vs 16 KB packets;
    2 KB packets collapse to ~54 GB/s. 2000-col DMA tiles (8 KB rows)
    are optimal: 16 KB tiles showed +1% rate but pair-granularity deps
    idle the engines in waves. The sync HWDGE queue only: gpsimd
    triggers take the slow SWDGE path, and splitting rows onto the
    Activation HWDGE queue halves total rate (shared engine set) or
    desyncs the 16-engine round-robin into a serial straggler.
  - DVE: f32 subtract 2.24us/2000col, 16-bit tensor_scalar/mult 0.68us,
    tensor_reduce 2.24us (no 16-bit speedup - only used on tiny tails).
    scalar_tensor_tensor+accum also runs 1x; tensor_scalar with
    accum_out silently drops op1 - both avoided.
  - ACT: 1.95us/2000col pass + 0.28us ACCUM_READ per accumulate.
  - PE: 0.62us per 500-col ones^T row-sum chunk.
  - Fixed tax inside exec_time: ~2.3us first-memset -> first HBM byte
    and ~9.4us NEFF semaphore teardown after the output lands.
  - Mid-stream ops that wait on other engines (e.g. PSUM drains gated
    on a chain close) head-of-line-block their queue and cascade into
    pool backpressure - all cross-engine-gated work stays at the end.

Engine split per 2000-col tile (4.65us arrival budget): DVE does
subtract, in-place |e| (u16 mask), m = relu(|e|-5), plus in-place m*m on
5 tiles; ACT squares |e| everywhere and m on the other tiles; PE row-sums
|e| chunks (S1) and the DVE-squared m^2 chunks into separate PSUM banks.
Duties ~83%/82%/71%. Both long PSUM chains close at tile 14 so their
[1,500] reduces run overlapped with the tail; the tail (1000+500+500
cols from two 1000-col DMA pairs) keeps the post-last-byte chain short.
"""

import numpy as np

import concourse.bass as bass
import concourse.bacc as bacc
import concourse.mybir as mybir
from concourse.tile import TileContext
from concourse.bass_utils import run_bass_kernel_spmd

P = 128
COLS = 32000  # 160*160*160 / 128
DELTA = 5.0
N_CORES = 8
N_ELEM = float(P * COLS)
CHUNK = 500  # PE reduction column-chunk (PSUM bank limit 512 f32)

F32 = mybir.dt.float32
U16 = mybir.dt.uint16
BF16 = mybir.dt.bfloat16
ALU = mybir.AluOpType
ACTF = mybir.ActivationFunctionType
AX = mybir.AxisListType


def build():
    dma_tiles = [2000] * 15 + [1000, 1000]
    assert sum(dma_tiles) == COLS
    # compute tiles: (dma_idx, col offset within dma tile, width)
    compute = [(di, 0, 2000) for di in range(15)]
    compute += [(15, 0, 1000), (16, 0, 500), (16, 500, 500)]
    n_ct = len(compute)
    n_main = 15  # tiles 0..14 feed the "main" PSUM chains
    dve_m2 = {2, 5, 8, 11, 14}  # m^2 via DVE in-place mult + PE row-sum
    mm_main = sum(max(1, f // CHUNK) for _, _, f in compute[:n_main])
    mm_tail = sum(max(1, f // CHUNK) for _, _, f in compute[n_main:])
    mm2_tot = sum(
        max(1, f // CHUNK) for t, (_, _, f) in enumerate(compute) if t in dve_m2
    )
    n_act = sum(1 for t in range(n_ct) if t not in dve_m2)

    nc = bacc.Bacc(
        "TRN2",
        target_bir_lowering=False,
        debug=False,
        enable_asserts=False,
        num_devices=N_CORES,
    )
    a_ext = nc.dram_tensor("y_pred_logits", [P, COLS], F32, kind="ExternalInput")
    b_ext = nc.dram_tensor("y_true", [P, COLS], F32, kind="ExternalInput")
    out_ext = nc.dram_tensor("out", [1, 5], F32, kind="ExternalOutput")

    with TileContext(nc) as tc:
        with (
            tc.tile_pool(name="io", bufs=7) as io_pool,
            tc.tile_pool(name="work", bufs=5) as work_pool,
            tc.tile_pool(name="acc", bufs=1) as acc_pool,
            tc.tile_pool(name="psum", bufs=1, space="PSUM") as psum_pool,
        ):
            sums_sq = acc_pool.tile([P, n_ct], F32)
            sums_d2a = acc_pool.tile([P, max(n_act, 1)], F32)
            fin = acc_pool.tile([P, 5], F32)
            fin2 = acc_pool.tile([1, 5], F32)
            scr_sq = acc_pool.tile([P, 2000], BF16)
            scr_d2 = acc_pool.tile([P, 2000], BF16)
            ones_bf = acc_pool.tile([P, 1], BF16)
            ones_f = acc_pool.tile([P, 1], F32)
            nc.vector.memset(ones_bf[:], 1.0)
            nc.vector.memset(ones_f[:], 1.0)
            nc.vector.memset(fin[:], 0.0)
            psum_ae = psum_pool.tile([1, CHUNK], F32)   # S1, tiles 0..14
            psum_d2 = psum_pool.tile([1, CHUNK], F32)   # SR_dve (all in 0..14)
            psum_tl = psum_pool.tile([1, CHUNK], F32)   # S1, tail tiles
            ps2 = psum_pool.tile([1, 5], F32)

            io_tiles = {}
            col = 0
            for di, df in enumerate(dma_tiles):
                a = io_pool.tile([P, df], F32, tag="a")
                b = io_pool.tile([P, df], F32, tag="b")
                sl = slice(col, col + df)
                col += df
                nc.sync.dma_start(out=a[:], in_=a_ext[:, sl])
                nc.sync.dma_start(out=b[:], in_=b_ext[:, sl])
                io_tiles[di] = (a, b)
            assert col == COLS

            mm_i = 0
            mm2_i = 0
            mmt_i = 0
            am_i = 0
            for t, (di, off, f) in enumerate(compute):
                a, b = io_tiles[di]
                csl = slice(off, off + f)
                e = work_pool.tile([P, f], BF16, tag="e")
                m = work_pool.tile([P, f], BF16, tag="m")
                # e = a - b (bf16 out: unbiased rounding, ~1e-5 rel err
                # on the final loss, far under the 2e-2 gate)
                nc.vector.tensor_tensor(e[:], a[:, csl], b[:, csl], ALU.subtract)
                # |e| in place via u16 mask (2x 16-bit mode)
                nc.vector.tensor_scalar(
                    e.bitcast(U16)[:], e.bitcast(U16)[:],
                    0x7FFF, None, ALU.bitwise_and,
                )
                # m = max(|e|,5) - 5 == relu(|e|-5)
                nc.vector.tensor_scalar(
                    m[:], e[:], DELTA, -DELTA, ALU.max, ALU.add
                )
                # S2 partial: ACT Square(|e|) + row-accumulate
                nc.scalar.activation(
                    scr_sq[:, 0:f], e[:], ACTF.Square,
                    accum_out=sums_sq[:, t : t + 1],
                )
                if t in dve_m2:
                    nc.vector.tensor_tensor(m[:], m[:], m[:], ALU.mult)
                    for c in range(f // CHUNK):
                        nc.tensor.matmul(
                            psum_d2[0:1, 0:CHUNK], ones_bf[:, 0:1],
                            m[:, c * CHUNK : (c + 1) * CHUNK],
                            start=(mm2_i == 0), stop=(mm2_i == mm2_tot - 1),
                        )
                        mm2_i += 1
                else:
                    nc.scalar.activation(
                        scr_d2[:, 0:f], m[:], ACTF.Square,
                        accum_out=sums_d2a[:, am_i : am_i + 1],
                    )
                    am_i += 1
                # S1 partial: ones^T @ |e| chunks; tail tiles use their
                # own bank so the main chain's reduce overlaps the tail
                for c in range(max(1, f // CHUNK)):
                    w = min(CHUNK, f - c * CHUNK)
                    if t < n_main:
                        nc.tensor.matmul(
                            psum_ae[0:1, 0:w], ones_bf[:, 0:1],
                            e[:, c * CHUNK : c * CHUNK + w],
                            start=(mm_i == 0), stop=(mm_i == mm_main - 1),
                        )
                        mm_i += 1
                    else:
                        nc.tensor.matmul(
                            psum_tl[0:1, 0:w], ones_bf[:, 0:1],
                            e[:, c * CHUNK : c * CHUNK + w],
                            start=(mmt_i == 0), stop=(mmt_i == mm_tail - 1),
                        )
                        mmt_i += 1
            assert mm_i == mm_main and mmt_i == mm_tail
            assert mm2_i == mm2_tot and am_i == n_act

            # fin cols: 0=S2/part, 1=SR_act/part, 2=SR_dve (p0), 3=S1 main
            # (p0), 4=S1 tail (p0). Cols 2 and 3 overlap the tail tiles.
            nc.vector.reduce_sum(fin[0:1, 2:3], psum_d2[0:1, :], axis=AX.X)
            nc.vector.reduce_sum(fin[0:1, 3:4], psum_ae[0:1, :], axis=AX.X)
            nc.vector.reduce_sum(fin[:, 0:1], sums_sq[:], axis=AX.X)
            nc.vector.reduce_sum(fin[:, 1:2], sums_d2a[:], axis=AX.X)
            nc.vector.reduce_sum(fin[0:1, 4:5], psum_tl[0:1, :], axis=AX.X)
            # partition-collapse so the output is one 20 B DMA packet
            nc.tensor.matmul(ps2[0:1, 0:5], ones_f[:, 0:1], fin[:, 0:5],
                             start=True, stop=True)
            nc.vector.tensor_scalar(fin2[:], ps2[0:1, 0:5], 1.0, None, ALU.mult)
            nc.sync.dma_start(out=out_ext[:, :], in_=fin2[:])

    nc.compile()
    return nc


_NC_CACHE = {}


def _get_nc():
    if "nc" not in _NC_CACHE:
        _NC_CACHE["nc"] = build()
    return _NC_CACHE["nc"]


def kernel(y_pred_logits: np.ndarray, y_true: np.ndarray, _trace=False) -> np.ndarray:
    nc = _get_nc()
    a = np.ascontiguousarray(y_pred_logits, dtype=np.float32).reshape(N_CORES, P, COLS)
    b = np.ascontiguousarray(y_true, dtype=np.float32).reshape(N_CORES, P, COLS)
    in_maps = [{"y_pred_logits": a[i], "y_true": b[i]} for i in range(N_CORES)]
    # the fleet occasionally reports a transient NRT_EXEC_UNIT_UNRECOVERABLE
    # from a prior aborted run; it clears on retry
    last_err = None
    for attempt in range(3):
        try:
            r = run_bass_kernel_spmd(
                nc, in_maps, core_ids=list(range(N_CORES)), trace=_trace
            )
            break
        except Exception as exc:  # noqa: BLE001
            last_err = exc
            import time

            time.sleep(10.0)
    else:
        raise last_err
    per_sample = np.empty(N_CORES, dtype=np.float64)
    for i in range(N_CORES):
        s2, sra, srv, s1a, s1b = np.asarray(
            r.results[i]["out"], dtype=np.float64
        ).ravel()
        sr = sra + srv
        s1 = s1a + s1b
        l2 = s2 / N_ELEM
        l1 = s1 / N_ELEM
        huber = 0.5 * (s2 - sr) / N_ELEM
        per_sample[i] = l2 if (l2 <= 1.0 or l2 < l1 * l1) else huber
    out = np.float32(per_sample.mean()).reshape(())
    if _trace:
        return out, r
    return out
